# revision 1
# baseline (speedup 1.0000x reference)
"""Trainium2 Bass kernel for nn_ODEG_8942121911067 (gnn_message_passing).

Math (derived from the reference ODE block; Euler steps collapse to the
last one since f is recomputed from x_aug each iteration):

    out = relu(0.5*x_aug + 0.125*sigmoid(alpha)_i * (adj @ x_aug)
               + 0.25*S*R + 0.25*(x_aug @_t W2mix))

with x_aug = concat([x, zeros10], -1), S[b,n,t] = sum_f x_aug[b,n,t,f],
R[m] = sum_n ((w*clip(d,0,1)) @ w.T)[m,n], W2mix = (w2*clip(d2,0,1)) @ w2.T.

Device strategy (data-parallel over batch, 4 batches/core on 8 cores):
  - The node term and the 0.5*x term fold into one stationary matrix
    A = 0.125*diag(sigmoid(alpha)) @ adj + 0.5*I  (host-computed, 1 MB),
    giving one K=512 matmul per output tile on the PE (fp32r, full rate).
  - The temporal mix (contraction over T=24) and the rank-1 S*R term are
    layout-hostile to the PE (they need t on partitions while the node
    matmul needs nodes on partitions); they are tiny (<5% of FLOPs) and
    are folded host-side into a side tensor q, which the device adds
    during PSUM eviction on the DVE. ReLU runs on ACT. The kernel is
    memory-bound: ~43 MB of HBM traffic per core.
"""

import numpy as np

B, N, T, F = 32, 512, 24, 64
NUM_ZEROS = 10
FA = F + NUM_ZEROS  # 74
N_CORES = 8
BPC = B // N_CORES  # batches per core = 4
NT = N // 128  # node chunks = 4
NCH = (T * F) // 512  # moving-dim chunks of 512 = 3
TPC = 512 // F  # t-values per 512-chunk = 8

_CACHE = {}


def _build(matmul_dtype_name="float32r"):
    import concourse.mybir as mybir
    import concourse.tile as tile
    from concourse import bacc

    mm_dt = getattr(mybir.dt, matmul_dtype_name)
    f32 = mybir.dt.float32

    nc = bacc.Bacc("TRN2", target_bir_lowering=False, debug=False,
                   num_devices=N_CORES)
    x_d = nc.dram_tensor("xin", [BPC, N, T, F], mm_dt, kind="ExternalInput").ap()
    q_d = nc.dram_tensor("q", [BPC, N, T, FA], f32, kind="ExternalInput").ap()
    at_d = nc.dram_tensor("at", [N, N], mm_dt, kind="ExternalInput").ap()
    out_d = nc.dram_tensor("out", [BPC, N, T, FA], f32, kind="ExternalOutput").ap()

    with tile.TileContext(nc) as tc:
        with (
            tc.tile_pool(name="const", bufs=1) as cpool,
            tc.tile_pool(name="xp", bufs=2 * NT) as xpool,
            tc.tile_pool(name="qp", bufs=3) as qpool,
            tc.tile_pool(name="op", bufs=3) as opool,
            tc.tile_pool(name="ps", bufs=6, space="PSUM") as pspool,
        ):
            at_sb = []
            for kc in range(NT):
                a = cpool.tile([128, N], mm_dt, tag=f"at{kc}")
                nc.sync.dma_start(a[:], at_d[kc * 128:(kc + 1) * 128, :])
                at_sb.append(a)

            for b in range(BPC):
                xts = []
                for kc in range(NT):
                    xt = xpool.tile([128, T, F], mm_dt, tag="xt")
                    nc.sync.dma_start(xt[:], x_d[b, kc * 128:(kc + 1) * 128])
                    xts.append(xt.rearrange("p a b -> p (a b)"))
                for ic in range(NT):
                    qt = qpool.tile([128, T, FA], f32, tag="qt")
                    nc.sync.dma_start(qt[:], q_d[b, ic * 128:(ic + 1) * 128])
                    ot = opool.tile([128, T, FA], f32, tag="ot")
                    for nch in range(NCH):
                        ps = pspool.tile([128, 512], f32, tag="ps")
                        for kc in range(NT):
                            nc.tensor.matmul(
                                ps[:],
                                at_sb[kc][:, ic * 128:(ic + 1) * 128],
                                xts[kc][:, nch * 512:(nch + 1) * 512],
                                start=(kc == 0),
                                stop=(kc == NT - 1),
                            )
                        t0 = nch * TPC
                        nc.vector.scalar_tensor_tensor(
                            ot[:, t0:t0 + TPC, 0:F],
                            ps[:].rearrange("p (a b) -> p a b", a=TPC),
                            1.0,
                            qt[:, t0:t0 + TPC, 0:F],
                            mybir.AluOpType.mult,
                            mybir.AluOpType.add,
                        )
                    nc.vector.tensor_copy(ot[:, :, F:FA], qt[:, :, F:FA])
                    nc.scalar.activation(ot[:], ot[:],
                                         mybir.ActivationFunctionType.Relu)
                    nc.sync.dma_start(out_d[b, ic * 128:(ic + 1) * 128], ot[:])

    nc.compile()
    return nc


def _prep_host(x, adj, alpha, w, d, w2, d2):
    x = np.ascontiguousarray(x, np.float32)
    a = 1.0 / (1.0 + np.exp(-alpha.astype(np.float32)))
    A = 0.125 * a[:, None] * adj.astype(np.float32)
    A[np.arange(N), np.arange(N)] += 0.5
    at = np.ascontiguousarray(A.T)

    dc = np.clip(d.astype(np.float32), 0.0, 1.0)
    W = (w.astype(np.float32) * dc) @ w.astype(np.float32).T
    R = W.sum(axis=1)  # [FA]
    d2c = np.clip(d2.astype(np.float32), 0.0, 1.0)
    W2 = (w2.astype(np.float32) * d2c) @ w2.astype(np.float32).T  # [T,T]

    S = x.sum(axis=3)  # [B,N,T]
    # q = 0.25 * (x @_t W2) padded to FA  +  0.25 * S[...,None] * R
    q = np.empty((B, N, T, FA), np.float32)
    xt = np.matmul(x.transpose(0, 1, 3, 2), 0.25 * W2)  # [B,N,F,T]
    q[..., :F] = xt.transpose(0, 1, 3, 2)
    q[..., F:] = 0.0
    q += 0.25 * S[..., None] * R
    return x, q, at


def kernel(x, adj, alpha, w, d, w2, d2):
    from concourse.bass_utils import run_bass_kernel_spmd

    x, q, at = _prep_host(x, adj, alpha, w, d, w2, d2)

    if "nc" not in _CACHE:
        _CACHE["nc"] = _build()
    nc = _CACHE["nc"]

    in_maps = [
        {"xin": x[c * BPC:(c + 1) * BPC], "q": q[c * BPC:(c + 1) * BPC], "at": at}
        for c in range(N_CORES)
    ]
    res = run_bass_kernel_spmd(nc, in_maps, list(range(N_CORES)))
    out = np.concatenate([res.results[c]["out"] for c in range(N_CORES)], axis=0)
    return out


# revision 2
# speedup vs baseline: 1.2908x; 1.2908x over previous
"""Trainium2 Bass kernel for nn_ODEG_8942121911067 (gnn_message_passing).

Math (derived from the reference ODE block; the Euler loop collapses to
its last step since f is recomputed from x_aug every iteration):

    out = relu(0.5*x_aug + 0.125*sigmoid(alpha)_i * (adj @ x_aug)
               + 0.25*S*R + 0.25*(x_aug @_t W2mix))

with x_aug = concat([x, zeros10], -1), S[b,n,t] = sum_f x_aug[b,n,t,f],
R[m] = sum_n ((w*clip(d,0,1)) @ w.T)[m,n], W2mix = (w2*clip(d2,0,1)) @ w2.T.

Device strategy (data-parallel over batch, 4 batches/core on 8 cores):
  - The node-mixing term runs as one K=512 matmul per output tile on the
    PE with stationary A = 0.125*diag(sigmoid(alpha)) @ adj (host-built,
    0.5 MB). x and A travel as bf16: the adjacency term is ~1% of the
    output magnitude, so bf16 rounding there is ~1e-5 relative overall.
  - All precision-critical linear terms (0.5*x, the temporal T=24 mix,
    the rank-1 S*R term) are folded host-side into a fp32 side tensor q
    (they are <5% of FLOPs but layout-hostile to the PE), which the DVE
    adds during PSUM eviction. ReLU runs on ACT. Outputs stream from SP
    and ACT HWDGE rings. The kernel is memory-bound (~36 MB HBM/core).
"""

import numpy as np

B, N, T, F = 32, 512, 24, 64
NUM_ZEROS = 10
FA = F + NUM_ZEROS  # 74
N_CORES = 8
BPC = B // N_CORES  # batches per core = 4
NT = N // 128  # node chunks = 4
NCH = (T * F) // 512  # moving-dim chunks of 512 = 3
TPC = 512 // F  # t-values per 512-chunk = 8

_CACHE = {}


def _build():
    import concourse.mybir as mybir
    import concourse.tile as tile
    from concourse import bacc

    bf16 = mybir.dt.bfloat16
    f32 = mybir.dt.float32

    nc = bacc.Bacc("TRN2", target_bir_lowering=False, debug=False,
                   num_devices=N_CORES)
    x_d = nc.dram_tensor("xin", [BPC, N, T, F], bf16, kind="ExternalInput").ap()
    q_d = nc.dram_tensor("q", [BPC, N, T, FA], f32, kind="ExternalInput").ap()
    at_d = nc.dram_tensor("at", [N, N], bf16, kind="ExternalInput").ap()
    out_d = nc.dram_tensor("out", [BPC, N, T, FA], f32, kind="ExternalOutput").ap()

    with tile.TileContext(nc) as tc:
        with (
            tc.tile_pool(name="const", bufs=1) as cpool,
            tc.tile_pool(name="xp", bufs=2 * NT) as xpool,
            tc.tile_pool(name="qp", bufs=3) as qpool,
            tc.tile_pool(name="op", bufs=3) as opool,
            tc.tile_pool(name="ps", bufs=6, space="PSUM") as pspool,
        ):
            at_sb = []
            for kc in range(NT):
                a = cpool.tile([128, N], bf16, tag=f"at{kc}")
                nc.sync.dma_start(a[:], at_d[kc * 128:(kc + 1) * 128, :])
                at_sb.append(a)

            for b in range(BPC):
                xts = []
                for kc in range(NT):
                    xt = xpool.tile([128, T, F], bf16, tag="xt")
                    nc.sync.dma_start(xt[:], x_d[b, kc * 128:(kc + 1) * 128])
                    xts.append(xt.rearrange("p a b -> p (a b)"))
                for ic in range(NT):
                    qt = qpool.tile([128, T, FA], f32, tag="qt")
                    nc.sync.dma_start(qt[:], q_d[b, ic * 128:(ic + 1) * 128])
                    ot = opool.tile([128, T, FA], f32, tag="ot")
                    for nch in range(NCH):
                        ps = pspool.tile([128, 512], f32, tag="ps")
                        for kc in range(NT):
                            nc.tensor.matmul(
                                ps[:],
                                at_sb[kc][:, ic * 128:(ic + 1) * 128],
                                xts[kc][:, nch * 512:(nch + 1) * 512],
                                start=(kc == 0),
                                stop=(kc == NT - 1),
                            )
                        t0 = nch * TPC
                        nc.vector.scalar_tensor_tensor(
                            ot[:, t0:t0 + TPC, 0:F],
                            ps[:].rearrange("p (a b) -> p a b", a=TPC),
                            1.0,
                            qt[:, t0:t0 + TPC, 0:F],
                            mybir.AluOpType.mult,
                            mybir.AluOpType.add,
                        )
                    # real cols: relu in place; pad cols: relu(q) directly
                    nc.scalar.activation(ot[:, :, 0:F], ot[:, :, 0:F],
                                         mybir.ActivationFunctionType.Relu)
                    nc.scalar.activation(ot[:, :, F:FA], qt[:, :, F:FA],
                                         mybir.ActivationFunctionType.Relu)
                    eng = nc.sync if (b + ic) % 2 == 0 else nc.scalar
                    eng.dma_start(out_d[b, ic * 128:(ic + 1) * 128], ot[:])

    nc.compile()
    return nc


def _prep_host(x, adj, alpha, w, d, w2, d2):
    import ml_dtypes

    x = np.ascontiguousarray(x, np.float32)
    a = 1.0 / (1.0 + np.exp(-alpha.astype(np.float32)))
    A = 0.125 * a[:, None] * adj.astype(np.float32)
    at = np.ascontiguousarray(A.T, dtype=ml_dtypes.bfloat16)

    dc = np.clip(d.astype(np.float32), 0.0, 1.0)
    W = (w.astype(np.float32) * dc) @ w.astype(np.float32).T
    R = W.sum(axis=1)  # [FA]
    d2c = np.clip(d2.astype(np.float32), 0.0, 1.0)
    W2 = (w2.astype(np.float32) * d2c) @ w2.astype(np.float32).T  # [T,T]

    S = x.sum(axis=3)  # [B,N,T]
    # q = 0.5*x + 0.25*(x @_t W2) padded to FA  +  0.25 * S[...,None] * R
    q = np.empty((B, N, T, FA), np.float32)
    xt = np.matmul(x.transpose(0, 1, 3, 2), 0.25 * W2)  # [B,N,F,T]
    q[..., :F] = xt.transpose(0, 1, 3, 2)
    q[..., :F] += 0.5 * x
    q[..., F:] = 0.0
    q += 0.25 * S[..., None] * R
    xb = x.astype(ml_dtypes.bfloat16)
    return xb, q, at


def kernel(x, adj, alpha, w, d, w2, d2):
    from concourse.bass_utils import run_bass_kernel_spmd

    xb, q, at = _prep_host(x, adj, alpha, w, d, w2, d2)

    if "nc" not in _CACHE:
        _CACHE["nc"] = _build()
    nc = _CACHE["nc"]

    in_maps = [
        {"xin": xb[c * BPC:(c + 1) * BPC], "q": q[c * BPC:(c + 1) * BPC], "at": at}
        for c in range(N_CORES)
    ]
    res = run_bass_kernel_spmd(nc, in_maps, list(range(N_CORES)))
    out = np.concatenate([res.results[c]["out"] for c in range(N_CORES)], axis=0)
    return out
